# revision 20
# baseline (speedup 1.0000x reference)
"""FBPINN (16 subdomain MLPs over [0,1]^2, cosine partition-of-unity windows)
as a Trainium2 Bass kernel with MoE-style routing across 8 NeuronCores.

Key idea: each subdomain's window is exactly zero outside its support box
(xmin-TW, xmax+TW), so its MLP only needs to run on the ~20-42% of points
inside that box. The host routes: it gathers each subdomain's active points
into 1024-point blocks (~84 blocks total vs 256 dense), load-balances the
blocks across the 8 cores, and does the final window-weighted scatter/
normalize. The device runs the heavy part: per block, a 3-layer tanh MLP
(256 wide) plus the W3 contraction.

Engine split per block: TensorE does the layer matmuls — layer 0 in f32r
with the b0 bias folded in as a K=3 ones-row (mt pair packed in PE row
groups 0/32), hidden layers in bf16, the W3 contraction pair packed in PE
column groups 0/32 (host adds the two partial rows); ScalarE applies tanh
per 128-feature tile with the b1/b2 bias fused into the activation's bias
operand. Two blocks are pipelined stage-locked so PE and ACT overlap;
PSUM holds exactly two 4-bank accumulator tiles.
"""

import numpy as np
import ml_dtypes

import concourse.bacc as bacc
import concourse.mybir as mybir
import concourse.tile as tile
from concourse.bass_utils import run_bass_kernel_spmd

K, D, N, W, OUT_DIM = 16, 2, 16384, 256, 1
TW = 0.2
NCORES = 8
P = 128
CH = 1024          # points per block
HALF = 512         # f32r matmul moving-operand subchunk (one PSUM bank)
FT = W // P        # feature tiles per hidden layer (2)

F32 = mybir.dt.float32
F32R = mybir.dt.float32r
BF16 = mybir.dt.bfloat16
AF = mybir.ActivationFunctionType
ALU = mybir.AluOpType
BF16NP = ml_dtypes.bfloat16


def _build_program(nblk):
    nc = bacc.Bacc("TRN2", target_bir_lowering=False, debug=False,
                   num_devices=NCORES)

    xad = nc.dram_tensor("XA", [6, nblk * CH], F32R, kind="ExternalInput")
    w0d = nc.dram_tensor("W0S", [6, nblk * P], F32R, kind="ExternalInput")
    w1d = nc.dram_tensor("W1S", [P, nblk * FT * FT, P], BF16, kind="ExternalInput")
    b1d = nc.dram_tensor("B1S", [P, nblk * FT], F32, kind="ExternalInput")
    w2d = nc.dram_tensor("W2S", [P, nblk * FT * FT, P], BF16, kind="ExternalInput")
    b2d = nc.dram_tensor("B2S", [P, nblk * FT], F32, kind="ExternalInput")
    w3d = nc.dram_tensor("W3S", [P, nblk * FT], BF16, kind="ExternalInput")
    outd = nc.dram_tensor("OUT", [nblk, 2, CH], F32, kind="ExternalOutput")

    with tile.TileContext(nc) as tc:
        with (
            tc.tile_pool(name="xin", bufs=nblk) as xin,
            tc.tile_pool(name="wgt", bufs=nblk) as wgt,
            tc.tile_pool(name="hbuf", bufs=8) as hbuf,
            tc.tile_pool(name="stage", bufs=4) as stage,
            tc.tile_pool(name="psum", bufs=4, space="PSUM") as psum,
        ):
            # consolidated input tiles; per-block DMA slices (block-major,
            # block 0 first so it unblocks immediately). Sub-tile dep
            # tracking scopes each block's matmuls to its own slice. Big
            # hidden weights stream on the gpsimd queue, rest on sync.
            # flat SBUF mirrors of the DRAM tensors: contiguous runs per
            # partition keep the DMA descriptor count (and the ~0.7us
            # serial issue per dma_start) low. Each tensor moves in two
            # chunks so the first pairs' data lands quickly while the
            # bulk streams in behind.
            xa = xin.tile([35, nblk * CH], F32R, tag="xa", bufs=1)
            w0 = wgt.tile([35, nblk * P], F32R, tag="w0", bufs=1)
            b1 = wgt.tile([P, nblk * FT], F32, tag="b1", bufs=1)
            b2 = wgt.tile([P, nblk * FT], F32, tag="b2", bufs=1)
            w3 = wgt.tile([P, nblk * FT], BF16, tag="w3", bufs=1)
            w1 = wgt.tile([P, nblk * FT * FT, P], BF16, tag="w1", bufs=1)
            w2 = wgt.tile([P, nblk * FT * FT, P], BF16, tag="w2", bufs=1)
            # the first pair's x lives in its own small tile so its
            # dependency closes as soon as its two tiny DMAs land; the
            # remaining blocks stream into the big tile behind it.
            s = min(2, nblk)
            xa0 = xin.tile([35, s * CH], F32R, tag="xa0", bufs=1)
            nc.sync.dma_start(xa0[0:3, :], xad[0:3, :s * CH])
            nc.sync.dma_start(xa0[32:35, :], xad[3:6, :s * CH])
            nc.sync.dma_start(w0[0:3, :], w0d[0:3, :])
            nc.sync.dma_start(w0[32:35, :], w0d[3:6, :])
            w = min(4, nblk)
            nc.gpsimd.dma_start(w1[:, :w * FT * FT, :], w1d[:, :w * FT * FT, :])
            nc.gpsimd.dma_start(w2[:, :w * FT * FT, :], w2d[:, :w * FT * FT, :])
            nc.scalar.dma_start(b1[:], b1d[:])
            nc.scalar.dma_start(b2[:], b2d[:])
            nc.gpsimd.dma_start(w3[:], w3d[:])
            if s < nblk:
                nc.scalar.dma_start(xa[0:3, s * CH:], xad[0:3, s * CH:])
                nc.scalar.dma_start(xa[32:35, s * CH:], xad[3:6, s * CH:])
            if w < nblk:
                nc.gpsimd.dma_start(w1[:, w * FT * FT:, :], w1d[:, w * FT * FT:, :])
                nc.gpsimd.dma_start(w2[:, w * FT * FT:, :], w2d[:, w * FT * FT:, :])

            def l0_mms(b, mt):
                # layer 0: K=3 f32r (two normalized coords + ones row
                # carrying b0); the mt pair lands in PE row groups 0/32
                # so the two units run concurrently.
                r0 = 32 * mt
                xsrc = xa0 if b < 2 else xa
                pt = psum.tile([P, CH], F32, tag="mm")
                for j in range(CH // HALF):
                    js = slice(j * HALF, (j + 1) * HALF)
                    nc.tensor.matmul(
                        pt[:, js], w0[r0:r0 + 3, b * P:(b + 1) * P],
                        xsrc[r0:r0 + 3, b * CH + j * HALF:b * CH + (j + 1) * HALF],
                        start=True, stop=True, tile_position=(r0, 0))
                return pt

            def hidden_mms(b, wl, mt, h):
                # bf16 weights, weight-major loops; the b1/b2 bias is
                # applied for free by the tanh activation's bias operand.
                pt = psum.tile([P, CH], F32, tag="mm")
                for ct in range(FT):
                    for j in range(CH // HALF):
                        js = slice(j * HALF, (j + 1) * HALF)
                        nc.tensor.matmul(
                            pt[:, js], wl[:, b * FT * FT + mt * FT + ct, :],
                            h[ct][:, js],
                            start=(ct == 0), stop=(ct == FT - 1),
                        )
                return pt

            def w3_mms(b, h):
                # the two ct tiles land in PE column groups 0/32 and run
                # concurrently; the host adds the two partial rows.
                pt = psum.tile([P, CH], F32, tag="mm")
                for j in range(CH // HALF):
                    js = slice(j * HALF, (j + 1) * HALF)
                    for ct, c0 in ((0, 0), (1, 32)):
                        nc.tensor.matmul(
                            pt[c0:c0 + 1, js], w3[:, b * FT + ct:b * FT + ct + 1],
                            h[ct][:, js],
                            start=True, stop=True, tile_position=(0, c0),
                        )
                return pt

            def act(pt, bl, b, mt):
                h = hbuf.tile([P, CH], BF16, tag="h")
                bias = bl[:, b * FT + mt:b * FT + mt + 1] if bl is not None else 0.0
                nc.scalar.activation(h[:], pt[:], AF.Tanh, bias=bias)
                return h

            # software pipeline: two blocks in flight, stage-interleaved;
            # PSUM runs as four independent 2-bank per-mt units so the next
            # pair's layer 0 overlaps the previous pair's W3/evacuation.
            for p0 in range(0, nblk, 2):
                blks = [b for b in (p0, p0 + 1) if b < nblk]
                ps = {(b, mt): l0_mms(b, mt) for b in blks for mt in range(FT)}
                h0 = {(b, mt): act(ps[b, mt], None, b, mt)
                      for b in blks for mt in range(FT)}
                ps = {(b, mt): hidden_mms(b, w1, mt, (h0[b, 0], h0[b, 1]))
                      for b in blks for mt in range(FT)}
                h1 = {(b, mt): act(ps[b, mt], b1, b, mt)
                      for b in blks for mt in range(FT)}
                ps = {(b, mt): hidden_mms(b, w2, mt, (h1[b, 0], h1[b, 1]))
                      for b in blks for mt in range(FT)}
                h2 = {(b, mt): act(ps[b, mt], b2, b, mt)
                      for b in blks for mt in range(FT)}
                pw = {b: w3_mms(b, (h2[b, 0], h2[b, 1])) for b in blks}
                for b in blks:
                    st = stage.tile([33, CH], F32, tag="out")
                    nc.vector.tensor_copy(st[:], pw[b][0:33, :])
                    nc.sync.dma_start(outd[b, 0], st[0:1, :])
                    nc.sync.dma_start(outd[b, 1], st[32:33, :])

    nc.compile()
    return nc


_PROGRAMS = {}
_LAST = {}


def _program(nblk=None):
    if nblk is None:
        nblk = _LAST.get("nblk", 11)
    if nblk not in _PROGRAMS:
        _PROGRAMS[nblk] = _build_program(nblk)
    return _PROGRAMS[nblk]


def _route(x, xmins, xmaxs):
    """Blocks of active points per subdomain + their window weights."""
    x64 = x.astype(np.float64)
    blocks = []  # (k, idx[int] padded to CH, real_len, wvals[real_len])
    for k in range(xmins.shape[0]):
        lo = xmins[k].astype(np.float64) - TW
        hi = xmaxs[k].astype(np.float64) + TW
        mask = np.all((x64 > lo) & (x64 < hi), axis=1)
        idx = np.nonzero(mask)[0].astype(np.int64)
        if idx.size == 0:
            continue
        t_l = np.clip((x64[idx] - lo) / (2.0 * TW), 0.0, 1.0)
        t_r = np.clip((hi - x64[idx]) / (2.0 * TW), 0.0, 1.0)
        wv = np.prod(0.25 * (1.0 - np.cos(np.pi * t_l))
                     * (1.0 - np.cos(np.pi * t_r)), axis=1)
        for c0 in range(0, idx.size, CH):
            ci = idx[c0:c0 + CH]
            real = ci.size
            if real < CH:
                ci = np.concatenate([ci, np.full(CH - real, idx[0])])
            blocks.append((k, ci, real, wv[c0:c0 + real]))
    return blocks


def _prep_in_maps(x, W0, b0, W1, b1, W2, b2, W3, b3, xmins, xmaxs):
    f32 = np.float32
    x = np.asarray(x, f32)
    center = ((xmins + xmaxs) * 0.5).astype(f32)
    scale = np.maximum((xmaxs - xmins) * 0.5, 1e-9).astype(f32)

    blocks = _route(x, xmins, xmaxs)
    per_core = [[] for _ in range(NCORES)]
    for j, blk in enumerate(blocks):
        per_core[j % NCORES].append(blk)
    nblk = max(len(c) for c in per_core)
    for c in per_core:
        while len(c) < nblk:
            k, ci, _, _ = c[0]
            c.append((k, ci, 0, np.zeros(0)))  # dummy, output ignored

    in_maps, meta = [], []
    for core in range(NCORES):
        xas = np.zeros((6, nblk * CH), f32)
        w0s = np.zeros((6, nblk * P), f32)
        w1s = np.zeros((P, nblk * FT * FT, P), f32)
        b1s = np.zeros((P, nblk * FT), f32)
        w2s = np.zeros((P, nblk * FT * FT, P), f32)
        b2s = np.zeros((P, nblk * FT), f32)
        w3s = np.zeros((P, nblk * FT), f32)
        cmeta = []
        for b, (k, ci, real, wv) in enumerate(per_core[core]):
            xn = (x[ci] - center[k]) / scale[k]       # [CH, 2]
            for r0 in (0, 3):
                xas[r0:r0 + 2, b * CH:(b + 1) * CH] = xn.T
                xas[r0 + 2, b * CH:(b + 1) * CH] = 1.0
            for mt in range(FT):
                r0 = 0 if mt == 0 else 3
                w0s[r0:r0 + 2, b * P:(b + 1) * P] = W0[k][:, mt * P:(mt + 1) * P]
                w0s[r0 + 2, b * P:(b + 1) * P] = b0[k][mt * P:(mt + 1) * P]
                b1s[:, b * FT + mt] = b1[k][mt * P:(mt + 1) * P]
                b2s[:, b * FT + mt] = b2[k][mt * P:(mt + 1) * P]
                w3s[:, b * FT + mt] = W3[k][mt * P:(mt + 1) * P, 0]
                for ct in range(FT):
                    w1s[:, b * FT * FT + mt * FT + ct, :] = (
                        W1[k][ct * P:(ct + 1) * P, mt * P:(mt + 1) * P])
                    w2s[:, b * FT * FT + mt * FT + ct, :] = (
                        W2[k][ct * P:(ct + 1) * P, mt * P:(mt + 1) * P])
            cmeta.append((k, ci, real, wv))
        in_maps.append({
            "XA": xas, "W0S": w0s,
            "W1S": w1s.astype(BF16NP), "B1S": b1s,
            "W2S": w2s.astype(BF16NP), "B2S": b2s,
            "W3S": w3s.astype(BF16NP),
        })
        meta.append(cmeta)

    _LAST.update(nblk=nblk, meta=meta, b3=np.asarray(b3, np.float64))
    return in_maps


def kernel(x, W0, b0, W1, b1, W2, b2, W3, b3, xmins, xmaxs):
    args = [np.asarray(a, np.float32) for a in
            (x, W0, b0, W1, b1, W2, b2, W3, b3, xmins, xmaxs)]
    in_maps = _prep_in_maps(*args)
    nc = _program(_LAST["nblk"])
    res = run_bass_kernel_spmd(nc, in_maps, list(range(NCORES)))

    n = x.shape[0]
    num = np.zeros(n, np.float64)
    den = np.zeros(n, np.float64)
    b3f = _LAST["b3"]
    for core in range(NCORES):
        out = np.asarray(res.results[core]["OUT"], np.float64)  # [nblk,2,CH]
        for b, (k, ci, real, wv) in enumerate(_LAST["meta"][core]):
            if real == 0:
                continue
            sub = out[b, 0, :real] + out[b, 1, :real] + b3f[k, 0]
            np.add.at(num, ci[:real], wv * sub)
            np.add.at(den, ci[:real], wv)
    result = (num / (den + 1e-9)).astype(np.float32)
    return result.reshape(n, OUT_DIM)


# revision 22
# speedup vs baseline: 1.1380x; 1.1380x over previous
"""FBPINN (16 subdomain MLPs over [0,1]^2, cosine partition-of-unity windows)
as a Trainium2 Bass kernel with MoE-style routing across 8 NeuronCores.

Key idea: each subdomain's window is exactly zero outside its support box
(xmin-TW, xmax+TW), so its MLP only needs to run on the ~20-42% of points
inside that box. The host routes: it gathers each subdomain's active points
into 1024-point blocks (plus one 512-point tail block per subdomain whose
remainder fits, to cut padding), load-balances the blocks across the 8
cores, and does the final window-weighted scatter/normalize. The device
runs the heavy part: per block, a 3-layer tanh MLP (256 wide) plus the W3
contraction.

Engine split per block: TensorE does the layer matmuls — layer 0 in f32r
with the b0 bias folded in as a K=3 ones-row (mt pair packed in PE row
groups 0/32), hidden layers in bf16, the W3 contraction pair packed in PE
column groups 0/32 (host adds the two partial rows); ScalarE applies tanh
per 128-feature tile with the b1/b2 bias fused into the activation's bias
operand. Two blocks are pipelined stage-locked so PE and ACT overlap;
PSUM runs as four independent 2-bank accumulator units.
"""

import numpy as np
import ml_dtypes

import concourse.bacc as bacc
import concourse.mybir as mybir
import concourse.tile as tile
from concourse.bass_utils import run_bass_kernel_spmd

K, D, N, W, OUT_DIM = 16, 2, 16384, 256, 1
TW = 0.2
NCORES = 8
P = 128
CH = 1024          # points per full block
HCH = 512          # points per half (tail) block
HALF = 512         # matmul moving-operand subchunk (one PSUM bank)
FT = W // P        # feature tiles per hidden layer (2)

F32 = mybir.dt.float32
F32R = mybir.dt.float32r
BF16 = mybir.dt.bfloat16
AF = mybir.ActivationFunctionType
ALU = mybir.AluOpType
BF16NP = ml_dtypes.bfloat16


def _xoff(i, nf):
    return i * CH if i < nf else nf * CH + (i - nf) * HCH


def _build_program(nf, nh):
    nblk = nf + nh
    xcols = nf * CH + nh * HCH
    nc = bacc.Bacc("TRN2", target_bir_lowering=False, debug=False,
                   num_devices=NCORES)

    xad = nc.dram_tensor("XA", [6, xcols], F32R, kind="ExternalInput")
    w0d = nc.dram_tensor("W0S", [6, nblk * P], F32R, kind="ExternalInput")
    w1d = nc.dram_tensor("W1S", [P, nblk * FT * FT, P], BF16, kind="ExternalInput")
    b1d = nc.dram_tensor("B1S", [P, nblk * FT], F32, kind="ExternalInput")
    w2d = nc.dram_tensor("W2S", [P, nblk * FT * FT, P], BF16, kind="ExternalInput")
    b2d = nc.dram_tensor("B2S", [P, nblk * FT], F32, kind="ExternalInput")
    w3d = nc.dram_tensor("W3S", [P, nblk * FT], BF16, kind="ExternalInput")
    outd = nc.dram_tensor("OUT", [nblk, 2, CH], F32, kind="ExternalOutput")

    with tile.TileContext(nc) as tc:
        with (
            tc.tile_pool(name="xin", bufs=1) as xin,
            tc.tile_pool(name="wgt", bufs=1) as wgt,
            tc.tile_pool(name="hbuf", bufs=8) as hbuf,
            tc.tile_pool(name="stage", bufs=4) as stage,
            tc.tile_pool(name="psum", bufs=4, space="PSUM") as psum,
        ):
            # flat SBUF mirrors of the DRAM tensors: contiguous runs per
            # partition keep DMA descriptor counts (and the ~0.7us serial
            # issue per dma_start) low. The first pair's x lives in its
            # own small tile so its dependency closes as soon as its two
            # tiny DMAs land; the rest streams in behind on two queues.
            xa = xin.tile([35, xcols], F32R, tag="xa")
            w0 = wgt.tile([35, nblk * P], F32R, tag="w0")
            b1 = wgt.tile([P, nblk * FT], F32, tag="b1")
            b2 = wgt.tile([P, nblk * FT], F32, tag="b2")
            w3 = wgt.tile([P, nblk * FT], BF16, tag="w3")
            w1 = wgt.tile([P, nblk * FT * FT, P], BF16, tag="w1")
            w2 = wgt.tile([P, nblk * FT * FT, P], BF16, tag="w2")
            scols = min(2 * CH, xcols)
            xa0 = xin.tile([35, scols], F32R, tag="xa0")
            nc.sync.dma_start(xa0[0:3, :], xad[0:3, :scols])
            nc.sync.dma_start(xa0[32:35, :], xad[3:6, :scols])
            nc.sync.dma_start(w0[0:3, :], w0d[0:3, :])
            nc.sync.dma_start(w0[32:35, :], w0d[3:6, :])
            w = min(4, nblk)
            nc.gpsimd.dma_start(w1[:, :w * FT * FT, :], w1d[:, :w * FT * FT, :])
            nc.gpsimd.dma_start(w2[:, :w * FT * FT, :], w2d[:, :w * FT * FT, :])
            nc.sync.dma_start(b1[:], b1d[:])
            nc.sync.dma_start(b2[:], b2d[:])
            nc.gpsimd.dma_start(w3[:], w3d[:])
            if scols < xcols:
                nc.sync.dma_start(xa[0:3, scols:], xad[0:3, scols:])
                nc.sync.dma_start(xa[32:35, scols:], xad[3:6, scols:])
            if w < nblk:
                nc.gpsimd.dma_start(w1[:, w * FT * FT:, :], w1d[:, w * FT * FT:, :])
                nc.gpsimd.dma_start(w2[:, w * FT * FT:, :], w2d[:, w * FT * FT:, :])

            def l0_mms(b, cb, xo, mt):
                # layer 0: K=3 f32r (two normalized coords + ones row
                # carrying b0); the mt pair lands in PE row groups 0/32
                # so the two units run concurrently.
                r0 = 32 * mt
                xsrc = xa0 if xo + cb <= scols else xa
                pt = psum.tile([P, CH], F32, tag="mm")
                for j in range(cb // HALF):
                    js = slice(j * HALF, (j + 1) * HALF)
                    nc.tensor.matmul(
                        pt[:, js], w0[r0:r0 + 3, b * P:(b + 1) * P],
                        xsrc[r0:r0 + 3, xo + j * HALF:xo + (j + 1) * HALF],
                        start=True, stop=True, tile_position=(r0, 0))
                return pt

            def hidden_mms(b, cb, wl, mt, h):
                # bf16 weights, weight-major loops; the b1/b2 bias is
                # applied for free by the tanh activation's bias operand.
                pt = psum.tile([P, CH], F32, tag="mm")
                for ct in range(FT):
                    for j in range(cb // HALF):
                        js = slice(j * HALF, (j + 1) * HALF)
                        nc.tensor.matmul(
                            pt[:, js], wl[:, b * FT * FT + mt * FT + ct, :],
                            h[ct][:, js],
                            start=(ct == 0), stop=(ct == FT - 1),
                        )
                return pt

            def w3_mms(b, cb, h):
                # the two ct tiles land in PE column groups 0/32 and run
                # concurrently; the host adds the two partial rows.
                pt = psum.tile([P, CH], F32, tag="mm")
                for j in range(cb // HALF):
                    js = slice(j * HALF, (j + 1) * HALF)
                    for ct, c0 in ((0, 0), (1, 32)):
                        nc.tensor.matmul(
                            pt[c0:c0 + 1, js], w3[:, b * FT + ct:b * FT + ct + 1],
                            h[ct][:, js],
                            start=True, stop=True, tile_position=(0, c0),
                        )
                return pt

            def act(pt, cb, bl, b, mt):
                h = hbuf.tile([P, CH], BF16, tag="h")
                bias = bl[:, b * FT + mt:b * FT + mt + 1] if bl is not None else 0.0
                nc.scalar.activation(h[:, :cb], pt[:, :cb], AF.Tanh, bias=bias)
                return h

            # software pipeline: two blocks in flight, stage-interleaved;
            # PSUM runs as four independent 2-bank units so the next
            # pair's layer 0 overlaps the previous pair's W3/evacuation.
            binfo = [(i, CH if i < nf else HCH, _xoff(i, nf))
                     for i in range(nblk)]
            for p0 in range(0, nblk, 2):
                blks = binfo[p0:p0 + 2]
                ps = {(b, mt): l0_mms(b, cb, xo, mt)
                      for b, cb, xo in blks for mt in range(FT)}
                h0 = {(b, mt): act(ps[b, mt], cb, None, b, mt)
                      for b, cb, xo in blks for mt in range(FT)}
                ps = {(b, mt): hidden_mms(b, cb, w1, mt, (h0[b, 0], h0[b, 1]))
                      for b, cb, xo in blks for mt in range(FT)}
                h1 = {(b, mt): act(ps[b, mt], cb, b1, b, mt)
                      for b, cb, xo in blks for mt in range(FT)}
                ps = {(b, mt): hidden_mms(b, cb, w2, mt, (h1[b, 0], h1[b, 1]))
                      for b, cb, xo in blks for mt in range(FT)}
                h2 = {(b, mt): act(ps[b, mt], cb, b2, b, mt)
                      for b, cb, xo in blks for mt in range(FT)}
                pw = {b: w3_mms(b, cb, (h2[b, 0], h2[b, 1])) for b, cb, xo in blks}
                for b, cb, xo in blks:
                    st = stage.tile([33, CH], F32, tag="out")
                    nc.vector.tensor_copy(st[:, :cb], pw[b][0:33, :cb])
                    nc.sync.dma_start(outd[b, 0, :cb], st[0:1, :cb])
                    nc.sync.dma_start(outd[b, 1, :cb], st[32:33, :cb])

    nc.compile()
    return nc


_PROGRAMS = {}
_LAST = {}


def _program(key=None):
    if key is None:
        key = _LAST.get("key", (10, 1))
    if key not in _PROGRAMS:
        _PROGRAMS[key] = _build_program(*key)
    return _PROGRAMS[key]


def _route(x, xmins, xmaxs):
    """Blocks of active points per subdomain + their window weights."""
    x64 = x.astype(np.float64)
    blocks = []  # (k, idx padded to cb, real_len, wvals[real_len], cb)
    for k in range(xmins.shape[0]):
        lo = xmins[k].astype(np.float64) - TW
        hi = xmaxs[k].astype(np.float64) + TW
        mask = np.all((x64 > lo) & (x64 < hi), axis=1)
        idx = np.nonzero(mask)[0].astype(np.int64)
        if idx.size == 0:
            continue
        t_l = np.clip((x64[idx] - lo) / (2.0 * TW), 0.0, 1.0)
        t_r = np.clip((hi - x64[idx]) / (2.0 * TW), 0.0, 1.0)
        wv = np.prod(0.25 * (1.0 - np.cos(np.pi * t_l))
                     * (1.0 - np.cos(np.pi * t_r)), axis=1)
        for c0 in range(0, idx.size, CH):
            ci = idx[c0:c0 + CH]
            real = ci.size
            cb = HCH if real <= HCH else CH
            if real < cb:
                ci = np.concatenate([ci, np.full(cb - real, idx[0])])
            blocks.append((k, ci, real, wv[c0:c0 + real], cb))
    return blocks


def _prep_in_maps(x, W0, b0, W1, b1, W2, b2, W3, b3, xmins, xmaxs):
    f32 = np.float32
    x = np.asarray(x, f32)
    center = ((xmins + xmaxs) * 0.5).astype(f32)
    scale = np.maximum((xmaxs - xmins) * 0.5, 1e-9).astype(f32)

    blocks = _route(x, xmins, xmaxs)
    fulls = [blk for blk in blocks if blk[4] == CH]
    halves = [blk for blk in blocks if blk[4] == HCH]
    cores_f = [[] for _ in range(NCORES)]
    cores_h = [[] for _ in range(NCORES)]
    for j, blk in enumerate(fulls):
        cores_f[j % NCORES].append(blk)
    for j, blk in enumerate(halves):
        cores_h[j % NCORES].append(blk)
    nf = max(len(c) for c in cores_f)
    nh = max(len(c) for c in cores_h)
    proto = blocks[0]
    for cf, chh in zip(cores_f, cores_h):
        while len(cf) < nf:  # dummy full, output ignored
            ci = np.resize(proto[1], CH)
            cf.append((proto[0], ci, 0, np.zeros(0), CH))
        while len(chh) < nh:  # dummy half
            ci = np.resize(proto[1], HCH)
            chh.append((proto[0], ci, 0, np.zeros(0), HCH))
    per_core = [cf + chh for cf, chh in zip(cores_f, cores_h)]
    nblk = nf + nh
    xcols = nf * CH + nh * HCH

    in_maps, meta = [], []
    for core in range(NCORES):
        xas = np.zeros((6, xcols), f32)
        w0s = np.zeros((6, nblk * P), f32)
        w1s = np.zeros((P, nblk * FT * FT, P), f32)
        b1s = np.zeros((P, nblk * FT), f32)
        w2s = np.zeros((P, nblk * FT * FT, P), f32)
        b2s = np.zeros((P, nblk * FT), f32)
        w3s = np.zeros((P, nblk * FT), f32)
        cmeta = []
        for b, (k, ci, real, wv, cb) in enumerate(per_core[core]):
            xo = _xoff(b, nf)
            xn = (x[ci] - center[k]) / scale[k]       # [cb, 2]
            for r0 in (0, 3):
                xas[r0:r0 + 2, xo:xo + cb] = xn.T
                xas[r0 + 2, xo:xo + cb] = 1.0
            for mt in range(FT):
                r0 = 0 if mt == 0 else 3
                w0s[r0:r0 + 2, b * P:(b + 1) * P] = W0[k][:, mt * P:(mt + 1) * P]
                w0s[r0 + 2, b * P:(b + 1) * P] = b0[k][mt * P:(mt + 1) * P]
                b1s[:, b * FT + mt] = b1[k][mt * P:(mt + 1) * P]
                b2s[:, b * FT + mt] = b2[k][mt * P:(mt + 1) * P]
                w3s[:, b * FT + mt] = W3[k][mt * P:(mt + 1) * P, 0]
                for ct in range(FT):
                    w1s[:, b * FT * FT + mt * FT + ct, :] = (
                        W1[k][ct * P:(ct + 1) * P, mt * P:(mt + 1) * P])
                    w2s[:, b * FT * FT + mt * FT + ct, :] = (
                        W2[k][ct * P:(ct + 1) * P, mt * P:(mt + 1) * P])
            cmeta.append((k, ci, real, wv))
        in_maps.append({
            "XA": xas, "W0S": w0s,
            "W1S": w1s.astype(BF16NP), "B1S": b1s,
            "W2S": w2s.astype(BF16NP), "B2S": b2s,
            "W3S": w3s.astype(BF16NP),
        })
        meta.append(cmeta)

    _LAST.update(key=(nf, nh), meta=meta, b3=np.asarray(b3, np.float64))
    return in_maps


def kernel(x, W0, b0, W1, b1, W2, b2, W3, b3, xmins, xmaxs):
    args = [np.asarray(a, np.float32) for a in
            (x, W0, b0, W1, b1, W2, b2, W3, b3, xmins, xmaxs)]
    in_maps = _prep_in_maps(*args)
    nc = _program(_LAST["key"])
    res = run_bass_kernel_spmd(nc, in_maps, list(range(NCORES)))

    n = x.shape[0]
    num = np.zeros(n, np.float64)
    den = np.zeros(n, np.float64)
    b3f = _LAST["b3"]
    for core in range(NCORES):
        out = np.asarray(res.results[core]["OUT"], np.float64)  # [nblk,2,CH]
        for b, (k, ci, real, wv) in enumerate(_LAST["meta"][core]):
            if real == 0:
                continue
            sub = out[b, 0, :real] + out[b, 1, :real] + b3f[k, 0]
            np.add.at(num, ci[:real], wv * sub)
            np.add.at(den, ci[:real], wv)
    result = (num / (den + 1e-9)).astype(np.float32)
    return result.reshape(n, OUT_DIM)


# revision 23
# speedup vs baseline: 1.1468x; 1.0077x over previous
"""FBPINN (16 subdomain MLPs over [0,1]^2, cosine partition-of-unity windows)
as a Trainium2 Bass kernel with MoE-style routing across 8 NeuronCores.

Key idea: each subdomain's window is exactly zero outside its support box
(xmin-TW, xmax+TW), so its MLP only needs to run on the ~20-42% of points
inside that box. The host routes: it gathers each subdomain's active points
into 1024-point blocks (plus one 512-point tail block per subdomain whose
remainder fits, to cut padding), load-balances the blocks across the 8
cores, and does the final window-weighted scatter/normalize. The device
runs the heavy part: per block, a 3-layer tanh MLP (256 wide) plus the W3
contraction.

Engine split per block: TensorE does the layer matmuls — layer 0 in f32r
with the b0 bias folded in as a K=3 ones-row (mt pair packed in PE row
groups 0/32), hidden layers in bf16, the W3 contraction pair packed in PE
column groups 0/32 (host adds the two partial rows); ScalarE applies tanh
per 128-feature tile with the b1/b2 bias fused into the activation's bias
operand. Two blocks are pipelined stage-locked so PE and ACT overlap;
PSUM runs as four independent 2-bank accumulator units.
"""

import numpy as np
import ml_dtypes

import concourse.bacc as bacc
import concourse.mybir as mybir
import concourse.tile as tile
from concourse.bass_utils import run_bass_kernel_spmd

K, D, N, W, OUT_DIM = 16, 2, 16384, 256, 1
TW = 0.2
NCORES = 8
P = 128
CH = 1024          # points per full block
HCH = 512          # points per half (tail) block
HALF = 512         # matmul moving-operand subchunk (one PSUM bank)
FT = W // P        # feature tiles per hidden layer (2)

F32 = mybir.dt.float32
F32R = mybir.dt.float32r
BF16 = mybir.dt.bfloat16
AF = mybir.ActivationFunctionType
ALU = mybir.AluOpType
BF16NP = ml_dtypes.bfloat16


def _xoff(i, nf):
    return i * CH if i < nf else nf * CH + (i - nf) * HCH


def _build_program(nf, nh):
    nblk = nf + nh
    xcols = nf * CH + nh * HCH
    nc = bacc.Bacc("TRN2", target_bir_lowering=False, debug=False,
                   num_devices=NCORES)

    xad = nc.dram_tensor("XA", [6, xcols], F32R, kind="ExternalInput")
    w0d = nc.dram_tensor("W0S", [6, nblk * P], F32R, kind="ExternalInput")
    w1d = nc.dram_tensor("W1S", [P, nblk * FT * FT, P], BF16, kind="ExternalInput")
    b1d = nc.dram_tensor("B1S", [P, nblk * FT], F32, kind="ExternalInput")
    w2d = nc.dram_tensor("W2S", [P, nblk * FT * FT, P], BF16, kind="ExternalInput")
    b2d = nc.dram_tensor("B2S", [P, nblk * FT], F32, kind="ExternalInput")
    w3d = nc.dram_tensor("W3S", [P, nblk * FT], BF16, kind="ExternalInput")
    outd = nc.dram_tensor("OUT", [nblk, 2, CH], F32, kind="ExternalOutput")

    with tile.TileContext(nc) as tc:
        with (
            tc.tile_pool(name="xin", bufs=1) as xin,
            tc.tile_pool(name="wgt", bufs=1) as wgt,
            tc.tile_pool(name="hbuf", bufs=8) as hbuf,
            tc.tile_pool(name="stage", bufs=4) as stage,
            tc.tile_pool(name="psum", bufs=4, space="PSUM") as psum,
        ):
            # flat SBUF mirrors of the DRAM tensors: contiguous runs per
            # partition keep DMA descriptor counts (and the ~0.7us serial
            # issue per dma_start) low. The first pair's x lives in its
            # own small tile so its dependency closes as soon as its two
            # tiny DMAs land; the rest streams in behind on two queues.
            xa = xin.tile([35, xcols], F32R, tag="xa")
            w0 = wgt.tile([35, nblk * P], F32R, tag="w0")
            b1 = wgt.tile([P, nblk * FT], F32, tag="b1")
            b2 = wgt.tile([P, nblk * FT], F32, tag="b2")
            w3 = wgt.tile([P, nblk * FT], BF16, tag="w3")
            w1 = wgt.tile([P, nblk * FT * FT, P], BF16, tag="w1")
            w2 = wgt.tile([P, nblk * FT * FT, P], BF16, tag="w2")
            scols = min(2 * CH, xcols)
            xa0 = xin.tile([35, scols], F32R, tag="xa0")
            nc.sync.dma_start(xa0[0:3, :], xad[0:3, :scols])
            nc.sync.dma_start(xa0[32:35, :], xad[3:6, :scols])
            nc.sync.dma_start(w0[0:3, :], w0d[0:3, :])
            nc.sync.dma_start(w0[32:35, :], w0d[3:6, :])
            w = min(4, nblk)
            nc.gpsimd.dma_start(w1[:, :w * FT * FT, :], w1d[:, :w * FT * FT, :])
            nc.gpsimd.dma_start(w2[:, :w * FT * FT, :], w2d[:, :w * FT * FT, :])
            nc.gpsimd.dma_start(w3[:], w3d[:])

            def deferred_dmas():
                # emitted after the first pair's layer-0 matmuls so their
                # completion is not folded into those matmuls' wait.
                nc.sync.dma_start(b1[:], b1d[:])
                nc.sync.dma_start(b2[:], b2d[:])
                if scols < xcols:
                    nc.sync.dma_start(xa[0:3, scols:], xad[0:3, scols:])
                    nc.sync.dma_start(xa[32:35, scols:], xad[3:6, scols:])

            if w < nblk:
                nc.gpsimd.dma_start(w1[:, w * FT * FT:, :], w1d[:, w * FT * FT:, :])
                nc.gpsimd.dma_start(w2[:, w * FT * FT:, :], w2d[:, w * FT * FT:, :])

            def l0_mms(b, cb, xo, mt):
                # layer 0: K=3 f32r (two normalized coords + ones row
                # carrying b0); the mt pair lands in PE row groups 0/32
                # so the two units run concurrently.
                r0 = 32 * mt
                xsrc = xa0 if xo + cb <= scols else xa
                pt = psum.tile([P, CH], F32, tag="mm")
                for j in range(cb // HALF):
                    js = slice(j * HALF, (j + 1) * HALF)
                    nc.tensor.matmul(
                        pt[:, js], w0[r0:r0 + 3, b * P:(b + 1) * P],
                        xsrc[r0:r0 + 3, xo + j * HALF:xo + (j + 1) * HALF],
                        start=True, stop=True, tile_position=(r0, 0))
                return pt

            def hidden_mms(b, cb, wl, mt, h):
                # bf16 weights, weight-major loops; the b1/b2 bias is
                # applied for free by the tanh activation's bias operand.
                pt = psum.tile([P, CH], F32, tag="mm")
                for ct in range(FT):
                    for j in range(cb // HALF):
                        js = slice(j * HALF, (j + 1) * HALF)
                        nc.tensor.matmul(
                            pt[:, js], wl[:, b * FT * FT + mt * FT + ct, :],
                            h[ct][:, js],
                            start=(ct == 0), stop=(ct == FT - 1),
                        )
                return pt

            def w3_mms(b, cb, h):
                # the two ct tiles land in PE column groups 0/32 and run
                # concurrently; the host adds the two partial rows.
                pt = psum.tile([P, CH], F32, tag="mm")
                for j in range(cb // HALF):
                    js = slice(j * HALF, (j + 1) * HALF)
                    for ct, c0 in ((0, 0), (1, 32)):
                        nc.tensor.matmul(
                            pt[c0:c0 + 1, js], w3[:, b * FT + ct:b * FT + ct + 1],
                            h[ct][:, js],
                            start=True, stop=True, tile_position=(0, c0),
                        )
                return pt

            def act(pt, cb, bl, b, mt):
                h = hbuf.tile([P, CH], BF16, tag="h")
                bias = bl[:, b * FT + mt:b * FT + mt + 1] if bl is not None else 0.0
                nc.scalar.activation(h[:, :cb], pt[:, :cb], AF.Tanh, bias=bias)
                return h

            # software pipeline: two blocks in flight, stage-interleaved;
            # PSUM runs as four independent 2-bank units so the next
            # pair's layer 0 overlaps the previous pair's W3/evacuation.
            binfo = [(i, CH if i < nf else HCH, _xoff(i, nf))
                     for i in range(nblk)]
            for p0 in range(0, nblk, 2):
                blks = binfo[p0:p0 + 2]
                ps = {(b, mt): l0_mms(b, cb, xo, mt)
                      for b, cb, xo in blks for mt in range(FT)}
                if p0 == 0:
                    deferred_dmas()
                h0 = {(b, mt): act(ps[b, mt], cb, None, b, mt)
                      for b, cb, xo in blks for mt in range(FT)}
                ps = {(b, mt): hidden_mms(b, cb, w1, mt, (h0[b, 0], h0[b, 1]))
                      for b, cb, xo in blks for mt in range(FT)}
                h1 = {(b, mt): act(ps[b, mt], cb, b1, b, mt)
                      for b, cb, xo in blks for mt in range(FT)}
                ps = {(b, mt): hidden_mms(b, cb, w2, mt, (h1[b, 0], h1[b, 1]))
                      for b, cb, xo in blks for mt in range(FT)}
                h2 = {(b, mt): act(ps[b, mt], cb, b2, b, mt)
                      for b, cb, xo in blks for mt in range(FT)}
                pw = {b: w3_mms(b, cb, (h2[b, 0], h2[b, 1])) for b, cb, xo in blks}
                for b, cb, xo in blks:
                    st = stage.tile([33, CH], F32, tag="out")
                    nc.vector.tensor_copy(st[:, :cb], pw[b][0:33, :cb])
                    nc.sync.dma_start(outd[b, 0, :cb], st[0:1, :cb])
                    nc.sync.dma_start(outd[b, 1, :cb], st[32:33, :cb])

    nc.compile()
    return nc


_PROGRAMS = {}
_LAST = {}


def _program(key=None):
    if key is None:
        key = _LAST.get("key", (10, 1))
    if key not in _PROGRAMS:
        _PROGRAMS[key] = _build_program(*key)
    return _PROGRAMS[key]


def _route(x, xmins, xmaxs):
    """Blocks of active points per subdomain + their window weights."""
    x64 = x.astype(np.float64)
    blocks = []  # (k, idx padded to cb, real_len, wvals[real_len], cb)
    for k in range(xmins.shape[0]):
        lo = xmins[k].astype(np.float64) - TW
        hi = xmaxs[k].astype(np.float64) + TW
        mask = np.all((x64 > lo) & (x64 < hi), axis=1)
        idx = np.nonzero(mask)[0].astype(np.int64)
        if idx.size == 0:
            continue
        t_l = np.clip((x64[idx] - lo) / (2.0 * TW), 0.0, 1.0)
        t_r = np.clip((hi - x64[idx]) / (2.0 * TW), 0.0, 1.0)
        wv = np.prod(0.25 * (1.0 - np.cos(np.pi * t_l))
                     * (1.0 - np.cos(np.pi * t_r)), axis=1)
        for c0 in range(0, idx.size, CH):
            ci = idx[c0:c0 + CH]
            real = ci.size
            cb = HCH if real <= HCH else CH
            if real < cb:
                ci = np.concatenate([ci, np.full(cb - real, idx[0])])
            blocks.append((k, ci, real, wv[c0:c0 + real], cb))
    return blocks


def _prep_in_maps(x, W0, b0, W1, b1, W2, b2, W3, b3, xmins, xmaxs):
    f32 = np.float32
    x = np.asarray(x, f32)
    center = ((xmins + xmaxs) * 0.5).astype(f32)
    scale = np.maximum((xmaxs - xmins) * 0.5, 1e-9).astype(f32)

    blocks = _route(x, xmins, xmaxs)
    fulls = [blk for blk in blocks if blk[4] == CH]
    halves = [blk for blk in blocks if blk[4] == HCH]
    cores_f = [[] for _ in range(NCORES)]
    cores_h = [[] for _ in range(NCORES)]
    for j, blk in enumerate(fulls):
        cores_f[j % NCORES].append(blk)
    for j, blk in enumerate(halves):
        cores_h[j % NCORES].append(blk)
    nf = max(len(c) for c in cores_f)
    nh = max(len(c) for c in cores_h)
    proto = blocks[0]
    for cf, chh in zip(cores_f, cores_h):
        while len(cf) < nf:  # dummy full, output ignored
            ci = np.resize(proto[1], CH)
            cf.append((proto[0], ci, 0, np.zeros(0), CH))
        while len(chh) < nh:  # dummy half
            ci = np.resize(proto[1], HCH)
            chh.append((proto[0], ci, 0, np.zeros(0), HCH))
    per_core = [cf + chh for cf, chh in zip(cores_f, cores_h)]
    nblk = nf + nh
    xcols = nf * CH + nh * HCH

    in_maps, meta = [], []
    for core in range(NCORES):
        xas = np.zeros((6, xcols), f32)
        w0s = np.zeros((6, nblk * P), f32)
        w1s = np.zeros((P, nblk * FT * FT, P), f32)
        b1s = np.zeros((P, nblk * FT), f32)
        w2s = np.zeros((P, nblk * FT * FT, P), f32)
        b2s = np.zeros((P, nblk * FT), f32)
        w3s = np.zeros((P, nblk * FT), f32)
        cmeta = []
        for b, (k, ci, real, wv, cb) in enumerate(per_core[core]):
            xo = _xoff(b, nf)
            xn = (x[ci] - center[k]) / scale[k]       # [cb, 2]
            for r0 in (0, 3):
                xas[r0:r0 + 2, xo:xo + cb] = xn.T
                xas[r0 + 2, xo:xo + cb] = 1.0
            for mt in range(FT):
                r0 = 0 if mt == 0 else 3
                w0s[r0:r0 + 2, b * P:(b + 1) * P] = W0[k][:, mt * P:(mt + 1) * P]
                w0s[r0 + 2, b * P:(b + 1) * P] = b0[k][mt * P:(mt + 1) * P]
                b1s[:, b * FT + mt] = b1[k][mt * P:(mt + 1) * P]
                b2s[:, b * FT + mt] = b2[k][mt * P:(mt + 1) * P]
                w3s[:, b * FT + mt] = W3[k][mt * P:(mt + 1) * P, 0]
                for ct in range(FT):
                    w1s[:, b * FT * FT + mt * FT + ct, :] = (
                        W1[k][ct * P:(ct + 1) * P, mt * P:(mt + 1) * P])
                    w2s[:, b * FT * FT + mt * FT + ct, :] = (
                        W2[k][ct * P:(ct + 1) * P, mt * P:(mt + 1) * P])
            cmeta.append((k, ci, real, wv))
        in_maps.append({
            "XA": xas, "W0S": w0s,
            "W1S": w1s.astype(BF16NP), "B1S": b1s,
            "W2S": w2s.astype(BF16NP), "B2S": b2s,
            "W3S": w3s.astype(BF16NP),
        })
        meta.append(cmeta)

    _LAST.update(key=(nf, nh), meta=meta, b3=np.asarray(b3, np.float64))
    return in_maps


def kernel(x, W0, b0, W1, b1, W2, b2, W3, b3, xmins, xmaxs):
    args = [np.asarray(a, np.float32) for a in
            (x, W0, b0, W1, b1, W2, b2, W3, b3, xmins, xmaxs)]
    in_maps = _prep_in_maps(*args)
    nc = _program(_LAST["key"])
    res = run_bass_kernel_spmd(nc, in_maps, list(range(NCORES)))

    n = x.shape[0]
    num = np.zeros(n, np.float64)
    den = np.zeros(n, np.float64)
    b3f = _LAST["b3"]
    for core in range(NCORES):
        out = np.asarray(res.results[core]["OUT"], np.float64)  # [nblk,2,CH]
        for b, (k, ci, real, wv) in enumerate(_LAST["meta"][core]):
            if real == 0:
                continue
            sub = out[b, 0, :real] + out[b, 1, :real] + b3f[k, 0]
            np.add.at(num, ci[:real], wv * sub)
            np.add.at(den, ci[:real], wv)
    result = (num / (den + 1e-9)).astype(np.float32)
    return result.reshape(n, OUT_DIM)
